# revision 17
# baseline (speedup 1.0000x reference)
"""EdgeConv (ParticleNet-style) Trainium2 kernel, v5.

Per event (16/core), software-pipelined 3 deep:
  - keys[i, j] = c_i.c_j - 0.5|c_j|^2 via bf16 hi/lo-split matmul (10-row
    contraction, host-prepped l10/r10, ~fp32-exact) + 128-col -BIG diag MMs.
  - top-16 per row via DVE max8/match_replace/max_index (fp32).
  - edge order n = t*2048 + p'*16 + r  (center i = 128t + p', slot r).
  - idx u16 -> fp16 -> flat rep[0:1, 8192] (4 DMAs) -> doubling DMAs -> [128,
    8192]; one-hot OH[jlo, jt, n] = (rep == jlo + 128*jt) via DVE
    tensor_scalar is_equal (fp16 in, bf16 out, fast mode).
  - gather per 512-col PSUM segment: p-add (static identrep x pnT_t) then
    4 jt one-hot matmuls; scalar ACT Relu drains -> vgT bf16
    (= relu(p_i + v_j)).
  - layer 2: ops[o, i] = sum_r (W2/16)^T @ vgT_r + b2; cols already in
    natural i order.
"""

import numpy as np
import ml_dtypes

B, N, F = 128, 512, 32
K = 16
H, OUT = 128, 64
NCORES = 8
EV = B // NCORES
BIG = np.float32(1e30)
NE = K * N          # 8192 edges per event
CH = 2048           # chunk = one t-block of centers
SEG = 512           # PSUM-bank-sized matmul segment

_cache = {}


def _build_nc(n_ev=EV):
    import concourse.bass as bass
    import concourse.bacc as bacc
    import concourse.tile as tile
    import concourse.mybir as mybir
    from contextlib import ExitStack

    dt = mybir.dt
    AOT = mybir.AluOpType
    AF = mybir.ActivationFunctionType
    nc = bacc.Bacc("TRN2", target_bir_lowering=False, debug=False,
                   enable_asserts=False, num_devices=NCORES)

    xtb_d = nc.dram_tensor("xtb", [n_ev, F + 2, N], dt.bfloat16,
                           kind="ExternalInput")
    l10_d = nc.dram_tensor("l10", [n_ev, 10, N], dt.bfloat16,
                           kind="ExternalInput")
    r10_d = nc.dram_tensor("r10", [n_ev, 10, N], dt.bfloat16,
                           kind="ExternalInput")
    wv_d = nc.dram_tensor("wv", [F + 1, H], dt.bfloat16, kind="ExternalInput")
    wpp_d = nc.dram_tensor("wpp", [F, H], dt.bfloat16, kind="ExternalInput")
    w2b_d = nc.dram_tensor("w2b", [H, OUT], dt.bfloat16, kind="ExternalInput")
    wxb_d = nc.dram_tensor("wxb", [2, OUT], dt.bfloat16, kind="ExternalInput")
    diag_d = nc.dram_tensor("diag", [128, 4, 128], dt.bfloat16,
                            kind="ExternalInput")
    ident_d = nc.dram_tensor("ident", [128, 128], dt.bfloat16,
                             kind="ExternalInput")
    idrep_d = nc.dram_tensor("idrep", [128, CH], dt.bfloat16,
                             kind="ExternalInput")
    iotn_d = nc.dram_tensor("iotn", [128, 4], dt.float32, kind="ExternalInput")
    out_d = nc.dram_tensor("out", [n_ev, OUT, N], dt.float32,
                           kind="ExternalOutput")

    with tile.TileContext(nc) as tc, ExitStack() as ctx:
        cpool = ctx.enter_context(tc.tile_pool(name="consts", bufs=1))
        ident = cpool.tile([128, 128], dt.bfloat16)
        nc.sync.dma_start(ident[:], ident_d[:])
        diag = cpool.tile([128, 4, 128], dt.bfloat16)
        nc.sync.dma_start(diag[:], diag_d[:])
        wv = cpool.tile([F + 1, H], dt.bfloat16)
        nc.sync.dma_start(wv[:], wv_d[:])
        wpp = cpool.tile([F, H], dt.bfloat16)
        nc.sync.dma_start(wpp[:], wpp_d[:])
        w2b = cpool.tile([H, OUT], dt.bfloat16)
        nc.sync.dma_start(w2b[:], w2b_d[:])
        wxb = cpool.tile([2, OUT], dt.bfloat16)
        nc.sync.dma_start(wxb[:], wxb_d[:])
        idrep = cpool.tile([128, CH], dt.bfloat16)
        nc.sync.dma_start(idrep[:], idrep_d[:])
        iotn = cpool.tile([128, 4], dt.float32)
        nc.sync.dma_start(iotn[:], iotn_d[:])
        ones2 = cpool.tile([2, N], dt.bfloat16)
        nc.gpsimd.memset(ones2[:], 1.0)

        xtb_pool = ctx.enter_context(tc.tile_pool(name="xtb", bufs=3))
        l10_pool = ctx.enter_context(tc.tile_pool(name="l10", bufs=3))
        r10_pool = ctx.enter_context(tc.tile_pool(name="r10", bufs=3))
        kt_pool = ctx.enter_context(tc.tile_pool(name="kt", bufs=8))
        k2_pool = ctx.enter_context(tc.tile_pool(name="k2", bufs=2))
        vals_pool = ctx.enter_context(tc.tile_pool(name="vals", bufs=2))
        idxs_pool = ctx.enter_context(tc.tile_pool(name="idxs", bufs=2))
        idxf_pool = ctx.enter_context(tc.tile_pool(name="idxf", bufs=2))
        rep_pool = ctx.enter_context(tc.tile_pool(name="rep", bufs=3))
        vsb_pool = ctx.enter_context(tc.tile_pool(name="vsb", bufs=3))
        pn_pool = ctx.enter_context(tc.tile_pool(name="pn", bufs=3))
        oh_pool = ctx.enter_context(tc.tile_pool(name="oh", bufs=2))
        vgT_pool = ctx.enter_context(tc.tile_pool(name="vgT", bufs=2))
        osb_pool = ctx.enter_context(tc.tile_pool(name="osb", bufs=2))

        kps_pool = ctx.enter_context(
            tc.tile_pool(name="kps", bufs=2, space="PSUM"))
        vpn_pool = ctx.enter_context(
            tc.tile_pool(name="vpn", bufs=1, space="PSUM"))
        gps_pool = ctx.enter_context(
            tc.tile_pool(name="gps", bufs=3, space="PSUM"))
        ops_pool = ctx.enter_context(
            tc.tile_pool(name="ops", bufs=1, space="PSUM"))

        st = [None] * n_ev

        def stage_a(e):
            """Load host-prepped xtb / l10 / r10 for event e."""
            xtb = xtb_pool.tile([F + 2, N], dt.bfloat16)
            nc.sync.dma_start(xtb[:], xtb_d[e])
            l10 = l10_pool.tile([10, N], dt.bfloat16)
            nc.sync.dma_start(l10[:], l10_d[e])
            r10 = r10_pool.tile([10, N], dt.bfloat16)
            nc.sync.dma_start(r10[:], r10_d[e])
            st[e] = {"xtb": xtb, "l10": l10, "r10": r10}

        def stage_b(e):
            """keys/v/pn matmuls + PSUM drains for event e."""
            s = st[e]
            xtb, l10, r10 = s["xtb"], s["l10"], s["r10"]
            kts = []
            for t in range(4):
                kps = kps_pool.tile([128, N], dt.float32)
                nc.tensor.matmul(kps[:], l10[:, 128 * t:128 * (t + 1)],
                                 r10[:], start=True, stop=False)
                nc.tensor.matmul(kps[:, 128 * t:128 * (t + 1)], ident[:],
                                 diag[:, t, :], start=False, stop=True,
                                 skip_group_check=True)
                kt = kt_pool.tile([128, N], dt.float32)
                nc.scalar.activation(kt[:], kps[:], AF.Copy)
                kts.append(kt)
            vps = vpn_pool.tile([128, N], dt.float32, name="vps")
            for c in range(4):
                nc.tensor.matmul(vps[:, H * c:H * (c + 1)],
                                 xtb[0:F + 1, 128 * c:128 * (c + 1)],
                                 wv[:], start=True, stop=True)
            v_sb = vsb_pool.tile([128, 4, H], dt.bfloat16)
            nc.scalar.activation(v_sb[:].opt(), vps[:], AF.Copy)
            pnp = vpn_pool.tile([128, N], dt.float32, name="pnp")
            for c in range(4):
                nc.tensor.matmul(pnp[:, H * c:H * (c + 1)],
                                 xtb[0:F, 128 * c:128 * (c + 1)],
                                 wpp[:], start=True, stop=True)
            pn_sb = pn_pool.tile([128, 4, H], dt.bfloat16)
            nc.scalar.activation(pn_sb[:].opt(), pnp[:], AF.Copy)
            s.update(kts=kts, v_sb=v_sb, pn_sb=pn_sb)

        def stage_d(e):
            """Selection + idx assembly + rep replication for event e."""
            s = st[e]
            vals = vals_pool.tile([128, 16], dt.float32)
            idxs = idxs_pool.tile([128, 64], dt.uint16)
            for t in range(4):
                kt = s["kts"][t][:]
                k2 = k2_pool.tile([128, N], dt.float32)
                v0 = vals[:, 0:8]
                v1 = vals[:, 8:16]
                nc.vector.max(v0, kt)
                nc.vector.match_replace(k2[:], v0, kt, -float(BIG))
                nc.vector.max(v1, k2[:])
                nc.vector.max_index(idxs[:, 16 * t:16 * t + 8], v0, kt)
                nc.vector.max_index(idxs[:, 16 * t + 8:16 * t + 16],
                                    v1, k2[:])
            idxf = idxf_pool.tile([128, 64], dt.float16)
            nc.vector.tensor_copy(idxf[:], idxs[:])
            rep = rep_pool.tile([128, NE], dt.float16)
            # dst col n = t*2048 + p*16 + r  <-  idxf[p, 16t + r]
            with tc.high_priority():
                for t in range(4):
                    nc.sync.dma_start(
                        rep[0:1, CH * t:CH * (t + 1)].rearrange(
                            "q (p r) -> q p r", p=128, r=16),
                        idxf[:, 16 * t:16 * (t + 1)])
                w = 1
                while w < 128:
                    nc.sync.dma_start(rep[w:2 * w, :], rep[0:w, :])
                    w *= 2
            s.update(rep=rep)

        def stage_c(e, trange, vgT=None):
            """One-hot + gather + relu drain (+ layer 2 + out) for event e."""
            s = st[e]
            rep, v_sb, pn_sb = s["rep"], s["v_sb"], s["pn_sb"]
            if vgT is None:
                vgT = vgT_pool.tile([128, NE], dt.bfloat16)
            for t in trange:
                csl = slice(CH * t, CH * (t + 1))
                oh = oh_pool.tile([128, 4, CH], dt.bfloat16)
                with tc.high_priority():
                    for jt in range(4):
                        nc.vector.tensor_scalar(
                            oh[:, jt, :].opt(), rep[:, csl],
                            iotn[:, jt:jt + 1], None, op0=AOT.is_equal)
                for sg in range(CH // SEG):
                    gps = gps_pool.tile([128, SEG], dt.float32, name="gps")
                    nc.tensor.matmul(gps[:], pn_sb[:, t, :],
                                     idrep[:, SEG * sg:SEG * (sg + 1)],
                                     start=True, stop=False)
                    for jt in range(4):
                        nc.tensor.matmul(
                            gps[:], v_sb[:, jt, :],
                            oh[:, jt, SEG * sg:SEG * (sg + 1)].opt(),
                            start=False, stop=(jt == 3))
                    nc.scalar.activation(
                        vgT[:, CH * t + SEG * sg:CH * t + SEG * (sg + 1)],
                        gps[:], AF.Relu)
            if trange[-1] != 3:
                return vgT
            ops = ops_pool.tile([OUT, N], dt.float32)
            vg4 = vgT[:].rearrange("h (t p r) -> h r t p", t=4, p=128, r=16)
            for r in range(K):
                nc.tensor.matmul(ops[:], w2b[:], vg4[:, r].opt(),
                                 start=(r == 0), stop=False)
            nc.tensor.matmul(ops[:], wxb[:], ones2[:],
                             start=False, stop=True)
            osb = osb_pool.tile([OUT, N], dt.float32)
            nc.scalar.activation(osb[:], ops[:], AF.Copy)
            nc.sync.dma_start(out_d[e], osb[:])
            st[e] = None

        # 3-deep software pipeline; OH/gather split around selection so the
        # DVE feeds PE chunks 0-1, runs selection, then feeds chunks 2-3.
        stage_a(0)
        if n_ev > 1:
            stage_a(1)
        for e in range(n_ev):
            stage_b(e)
            if e + 2 < n_ev:
                stage_a(e + 2)
            vg = stage_c(e - 2, [0, 1]) if e >= 2 else None
            stage_d(e)
            if e >= 2:
                stage_c(e - 2, [2, 3], vg)
        for e in (n_ev - 2, n_ev - 1):
            vg = stage_c(e, [0, 1])
            stage_c(e, [2, 3], vg)

    nc.compile()
    return nc


def _prep_inputs(x, W1, b1, W2, b2):
    bf16 = ml_dtypes.bfloat16
    W1a = W1[0:F, :].astype(np.float64)
    W1b = W1[F:2 * F, :].astype(np.float64)
    Wp = W1a - W1b                            # center part
    Wv = W1[F:2 * F, :].astype(np.float32)    # neighbor part

    wv = np.zeros((F + 1, H), dtype=bf16)
    wv[0:F, :] = Wv.astype(bf16)
    wv[F, :] = b1.astype(bf16)

    wpp = Wp.astype(bf16)
    w2b = (W2.astype(np.float32) / np.float32(K)).astype(bf16)

    b2f = b2.astype(np.float32)
    b2hi = b2f.astype(bf16)
    b2lo = (b2f - b2hi.astype(np.float32)).astype(bf16)
    wxb = np.zeros((2, OUT), dtype=bf16)
    wxb[0, :] = b2hi
    wxb[1, :] = b2lo

    diag = np.zeros((128, 4, 128), dtype=bf16)
    for t in range(4):
        diag[np.arange(128), t, np.arange(128)] = bf16(-BIG)
    ident = np.eye(128, dtype=bf16)

    idrep = np.zeros((128, CH), dtype=bf16)
    for p in range(128):
        idrep[p, p * 16:(p + 1) * 16] = bf16(1.0)

    p128 = np.arange(128, dtype=np.float32)[:, None]
    jt4 = np.arange(4, dtype=np.float32)[None, :]
    iotn = (p128 + 128 * jt4).astype(np.float32)

    xt = np.ascontiguousarray(x.transpose(0, 2, 1).astype(np.float32))
    # host-side xtb (bf16 features + two ones rows)
    xtb = np.ones((B, F + 2, N), dtype=bf16)
    xtb[:, 0:F, :] = xt.astype(bf16)
    # host-side keys operands: hi/lo splits of coords and 0.5|c|^2
    c = xt[:, 0:2, :].astype(np.float32)                  # [B, 2, N]
    chi = c.astype(bf16)
    clo = (c - chi.astype(np.float32)).astype(bf16)
    sq = 0.5 * c * c
    sqhi = sq.astype(bf16)
    sqlo = (sq - sqhi.astype(np.float32)).astype(bf16)
    l10 = np.empty((B, 10, N), dtype=bf16)
    l10[:, 0:4, :] = bf16(-1.0)
    l10[:, 4, :] = chi[:, 0]
    l10[:, 5, :] = chi[:, 0]
    l10[:, 6, :] = clo[:, 0]
    l10[:, 7, :] = chi[:, 1]
    l10[:, 8, :] = chi[:, 1]
    l10[:, 9, :] = clo[:, 1]
    r10 = np.empty((B, 10, N), dtype=bf16)
    r10[:, 0, :] = sqhi[:, 0]
    r10[:, 1, :] = sqlo[:, 0]
    r10[:, 2, :] = sqhi[:, 1]
    r10[:, 3, :] = sqlo[:, 1]
    r10[:, 4, :] = chi[:, 0]
    r10[:, 5, :] = clo[:, 0]
    r10[:, 6, :] = chi[:, 0]
    r10[:, 7, :] = chi[:, 1]
    r10[:, 8, :] = clo[:, 1]
    r10[:, 9, :] = chi[:, 1]
    return xtb, l10, r10, wv, wpp, w2b, wxb, diag, ident, idrep, iotn


def _in_maps(xtb, l10, r10, wv, wpp, w2b, wxb, diag, ident, idrep, iotn):
    return [{
        "xtb": xtb[c * EV:(c + 1) * EV],
        "l10": l10[c * EV:(c + 1) * EV],
        "r10": r10[c * EV:(c + 1) * EV],
        "wv": wv, "wpp": wpp, "w2b": w2b, "wxb": wxb,
        "diag": diag, "ident": ident, "idrep": idrep, "iotn": iotn,
    } for c in range(NCORES)]


def kernel(x, W1, b1, W2, b2):
    from concourse.bass_utils import run_bass_kernel_spmd

    key = "nc"
    if key not in _cache:
        _cache[key] = _build_nc()
    nc = _cache[key]

    prepped = _prep_inputs(
        np.asarray(x), np.asarray(W1), np.asarray(b1),
        np.asarray(W2), np.asarray(b2))

    res = run_bass_kernel_spmd(nc, _in_maps(*prepped), list(range(NCORES)))
    outs = [res.results[c]["out"] for c in range(NCORES)]
    full = np.concatenate(outs, axis=0)
    return np.ascontiguousarray(full.transpose(0, 2, 1)).astype(np.float32)


# revision 18
# speedup vs baseline: 1.0527x; 1.0527x over previous
"""EdgeConv (ParticleNet-style) Trainium2 kernel, v5.

Per event (16/core), software-pipelined 3 deep:
  - keys[i, j] = c_i.c_j - 0.5|c_j|^2 via bf16 hi/lo-split matmul (10-row
    contraction, host-prepped l10/r10, ~fp32-exact) + 128-col -BIG diag MMs.
  - top-16 per row via DVE max8/match_replace/max_index (fp32).
  - edge order n = t*2048 + p'*16 + r  (center i = 128t + p', slot r).
  - idx u16 -> fp16 -> flat rep[0:1, 8192] (4 DMAs) -> doubling DMAs -> [128,
    8192]; one-hot OH[jlo, jt, n] = (rep == jlo + 128*jt) via DVE
    tensor_scalar is_equal (fp16 in, bf16 out, fast mode).
  - gather per 512-col PSUM segment: p-add (static identrep x pnT_t) then
    4 jt one-hot matmuls; scalar ACT Relu drains -> vgT bf16
    (= relu(p_i + v_j)).
  - layer 2: ops[o, i] = sum_r (W2/16)^T @ vgT_r + b2; cols already in
    natural i order.
"""

import numpy as np
import ml_dtypes

B, N, F = 128, 512, 32
K = 16
H, OUT = 128, 64
NCORES = 8
EV = B // NCORES
BIG = np.float32(1e30)
NE = K * N          # 8192 edges per event
CH = 2048           # chunk = one t-block of centers
SEG = 512           # PSUM-bank-sized matmul segment

_cache = {}


def _build_nc(n_ev=EV):
    import concourse.bass as bass
    import concourse.bacc as bacc
    import concourse.tile as tile
    import concourse.mybir as mybir
    from contextlib import ExitStack

    dt = mybir.dt
    AOT = mybir.AluOpType
    AF = mybir.ActivationFunctionType
    nc = bacc.Bacc("TRN2", target_bir_lowering=False, debug=False,
                   enable_asserts=False, num_devices=NCORES)

    xtb_d = nc.dram_tensor("xtb", [n_ev, F + 2, N], dt.bfloat16,
                           kind="ExternalInput")
    l10_d = nc.dram_tensor("l10", [n_ev, 10, N], dt.bfloat16,
                           kind="ExternalInput")
    r10_d = nc.dram_tensor("r10", [n_ev, 10, N], dt.bfloat16,
                           kind="ExternalInput")
    wv_d = nc.dram_tensor("wv", [F + 1, H], dt.bfloat16, kind="ExternalInput")
    wpp_d = nc.dram_tensor("wpp", [F, H], dt.bfloat16, kind="ExternalInput")
    w2b_d = nc.dram_tensor("w2b", [H, OUT], dt.bfloat16, kind="ExternalInput")
    wxb_d = nc.dram_tensor("wxb", [2, OUT], dt.bfloat16, kind="ExternalInput")
    diag_d = nc.dram_tensor("diag", [128, 4, 128], dt.bfloat16,
                            kind="ExternalInput")
    ident_d = nc.dram_tensor("ident", [128, 128], dt.bfloat16,
                             kind="ExternalInput")
    idrep_d = nc.dram_tensor("idrep", [128, CH], dt.bfloat16,
                             kind="ExternalInput")
    iotn_d = nc.dram_tensor("iotn", [128, 4], dt.float32, kind="ExternalInput")
    out_d = nc.dram_tensor("out", [n_ev, OUT, N], dt.float32,
                           kind="ExternalOutput")

    with tile.TileContext(nc) as tc, ExitStack() as ctx:
        cpool = ctx.enter_context(tc.tile_pool(name="consts", bufs=1))
        ident = cpool.tile([128, 128], dt.bfloat16)
        nc.sync.dma_start(ident[:], ident_d[:])
        diag = cpool.tile([128, 4, 128], dt.bfloat16)
        nc.sync.dma_start(diag[:], diag_d[:])
        wv = cpool.tile([F + 1, H], dt.bfloat16)
        nc.sync.dma_start(wv[:], wv_d[:])
        wpp = cpool.tile([F, H], dt.bfloat16)
        nc.sync.dma_start(wpp[:], wpp_d[:])
        w2b = cpool.tile([H, OUT], dt.bfloat16)
        nc.sync.dma_start(w2b[:], w2b_d[:])
        wxb = cpool.tile([2, OUT], dt.bfloat16)
        nc.sync.dma_start(wxb[:], wxb_d[:])
        idrep = cpool.tile([128, CH], dt.bfloat16)
        nc.sync.dma_start(idrep[:], idrep_d[:])
        iotn = cpool.tile([128, 4], dt.float32)
        nc.sync.dma_start(iotn[:], iotn_d[:])
        ones2 = cpool.tile([2, N], dt.bfloat16)
        nc.gpsimd.memset(ones2[:], 1.0)

        xtb_pool = ctx.enter_context(tc.tile_pool(name="xtb", bufs=3))
        l10_pool = ctx.enter_context(tc.tile_pool(name="l10", bufs=3))
        r10_pool = ctx.enter_context(tc.tile_pool(name="r10", bufs=3))
        kt_pool = ctx.enter_context(tc.tile_pool(name="kt", bufs=8))
        k2_pool = ctx.enter_context(tc.tile_pool(name="k2", bufs=2))
        vals_pool = ctx.enter_context(tc.tile_pool(name="vals", bufs=2))
        idxs_pool = ctx.enter_context(tc.tile_pool(name="idxs", bufs=2))
        idxf_pool = ctx.enter_context(tc.tile_pool(name="idxf", bufs=2))
        rep_pool = ctx.enter_context(tc.tile_pool(name="rep", bufs=3))
        vsb_pool = ctx.enter_context(tc.tile_pool(name="vsb", bufs=3))
        pn_pool = ctx.enter_context(tc.tile_pool(name="pn", bufs=3))
        oh_pool = ctx.enter_context(tc.tile_pool(name="oh", bufs=2))
        vgT_pool = ctx.enter_context(tc.tile_pool(name="vgT", bufs=2))
        osb_pool = ctx.enter_context(tc.tile_pool(name="osb", bufs=2))

        kps_pool = ctx.enter_context(
            tc.tile_pool(name="kps", bufs=2, space="PSUM"))
        vpn_pool = ctx.enter_context(
            tc.tile_pool(name="vpn", bufs=1, space="PSUM"))
        gps_pool = ctx.enter_context(
            tc.tile_pool(name="gps", bufs=3, space="PSUM"))
        ops_pool = ctx.enter_context(
            tc.tile_pool(name="ops", bufs=1, space="PSUM"))

        st = [None] * n_ev

        def stage_a(e):
            """Load host-prepped xtb / l10 / r10 for event e."""
            xtb = xtb_pool.tile([F + 2, N], dt.bfloat16)
            nc.sync.dma_start(xtb[:], xtb_d[e])
            l10 = l10_pool.tile([10, N], dt.bfloat16)
            nc.sync.dma_start(l10[:], l10_d[e])
            r10 = r10_pool.tile([10, N], dt.bfloat16)
            nc.sync.dma_start(r10[:], r10_d[e])
            st[e] = {"xtb": xtb, "l10": l10, "r10": r10}

        def stage_b(e):
            """keys/v/pn matmuls + PSUM drains for event e."""
            s = st[e]
            xtb, l10, r10 = s["xtb"], s["l10"], s["r10"]
            kts = []
            for t in range(4):
                kps = kps_pool.tile([128, N], dt.float32)
                nc.tensor.matmul(kps[:], l10[:, 128 * t:128 * (t + 1)],
                                 r10[:], start=True, stop=False)
                nc.tensor.matmul(kps[:, 128 * t:128 * (t + 1)], ident[:],
                                 diag[:, t, :], start=False, stop=True,
                                 skip_group_check=True)
                kt = kt_pool.tile([128, N], dt.float32)
                nc.scalar.activation(kt[:], kps[:], AF.Copy)
                kts.append(kt)
            vps = vpn_pool.tile([128, N], dt.float32, name="vps")
            for c in range(4):
                nc.tensor.matmul(vps[:, H * c:H * (c + 1)],
                                 xtb[0:F + 1, 128 * c:128 * (c + 1)],
                                 wv[:], start=True, stop=True)
            v_sb = vsb_pool.tile([128, 4, H], dt.bfloat16)
            nc.scalar.activation(v_sb[:].opt(), vps[:], AF.Copy)
            pnp = vpn_pool.tile([128, N], dt.float32, name="pnp")
            for c in range(4):
                nc.tensor.matmul(pnp[:, H * c:H * (c + 1)],
                                 xtb[0:F, 128 * c:128 * (c + 1)],
                                 wpp[:], start=True, stop=True)
            pn_sb = pn_pool.tile([128, 4, H], dt.bfloat16)
            nc.scalar.activation(pn_sb[:].opt(), pnp[:], AF.Copy)
            s.update(kts=kts, v_sb=v_sb, pn_sb=pn_sb)

        def stage_d(e):
            """Selection + idx assembly + rep replication for event e."""
            s = st[e]
            vals = vals_pool.tile([128, 16], dt.float32)
            idxs = idxs_pool.tile([128, 64], dt.uint16)
            for t in range(4):
                kt = s["kts"][t][:]
                k2 = k2_pool.tile([128, N], dt.float32)
                v0 = vals[:, 0:8]
                v1 = vals[:, 8:16]
                nc.vector.max(v0, kt)
                nc.vector.match_replace(k2[:], v0, kt, -float(BIG))
                nc.vector.max(v1, k2[:])
                nc.vector.max_index(idxs[:, 16 * t:16 * t + 8], v0, kt)
                nc.vector.max_index(idxs[:, 16 * t + 8:16 * t + 16],
                                    v1, k2[:])
            idxf = idxf_pool.tile([128, 64], dt.float16)
            nc.vector.tensor_copy(idxf[:], idxs[:])
            rep = rep_pool.tile([128, NE], dt.float16)
            # dst col n = t*2048 + p*16 + r  <-  idxf[p, 16t + r]
            for t in range(4):
                nc.sync.dma_start(
                    rep[0:1, CH * t:CH * (t + 1)].rearrange(
                        "q (p r) -> q p r", p=128, r=16),
                    idxf[:, 16 * t:16 * (t + 1)])
            w = 1
            while w < 128:
                nc.sync.dma_start(rep[w:2 * w, :], rep[0:w, :])
                w *= 2
            s.update(rep=rep)

        def stage_c(e, trange, vgT=None):
            """One-hot + gather + relu drain (+ layer 2 + out) for event e."""
            s = st[e]
            rep, v_sb, pn_sb = s["rep"], s["v_sb"], s["pn_sb"]
            if vgT is None:
                vgT = vgT_pool.tile([128, NE], dt.bfloat16)
            for t in trange:
                csl = slice(CH * t, CH * (t + 1))
                oh = oh_pool.tile([128, 4, CH], dt.bfloat16)
                with tc.high_priority():
                    for jt in range(4):
                        nc.vector.tensor_scalar(
                            oh[:, jt, :].opt(), rep[:, csl],
                            iotn[:, jt:jt + 1], None, op0=AOT.is_equal)
                for sg in range(CH // SEG):
                    gps = gps_pool.tile([128, SEG], dt.float32, name="gps")
                    nc.tensor.matmul(gps[:], pn_sb[:, t, :],
                                     idrep[:, SEG * sg:SEG * (sg + 1)],
                                     start=True, stop=False)
                    for jt in range(4):
                        nc.tensor.matmul(
                            gps[:], v_sb[:, jt, :],
                            oh[:, jt, SEG * sg:SEG * (sg + 1)].opt(),
                            start=False, stop=(jt == 3))
                    nc.scalar.activation(
                        vgT[:, CH * t + SEG * sg:CH * t + SEG * (sg + 1)],
                        gps[:], AF.Relu)
            if trange[-1] != 3:
                return vgT
            ops = ops_pool.tile([OUT, N], dt.float32)
            vg4 = vgT[:].rearrange("h (t p r) -> h r t p", t=4, p=128, r=16)
            for r in range(K):
                nc.tensor.matmul(ops[:], w2b[:], vg4[:, r].opt(),
                                 start=(r == 0), stop=False)
            nc.tensor.matmul(ops[:], wxb[:], ones2[:],
                             start=False, stop=True)
            osb = osb_pool.tile([OUT, N], dt.float32)
            nc.scalar.activation(osb[:], ops[:], AF.Copy)
            nc.sync.dma_start(out_d[e], osb[:])
            st[e] = None

        # 3-deep software pipeline; OH/gather split around selection so the
        # DVE feeds PE chunks 0-1, runs selection, then feeds chunks 2-3.
        stage_a(0)
        if n_ev > 1:
            stage_a(1)
        for e in range(n_ev):
            stage_b(e)
            if e + 2 < n_ev:
                stage_a(e + 2)
            vg = stage_c(e - 2, [0, 1]) if e >= 2 else None
            stage_d(e)
            if e >= 2:
                stage_c(e - 2, [2, 3], vg)
        for e in (n_ev - 2, n_ev - 1):
            vg = stage_c(e, [0, 1])
            stage_c(e, [2, 3], vg)

    nc.compile()
    return nc


def _prep_inputs(x, W1, b1, W2, b2):
    bf16 = ml_dtypes.bfloat16
    W1a = W1[0:F, :].astype(np.float64)
    W1b = W1[F:2 * F, :].astype(np.float64)
    Wp = W1a - W1b                            # center part
    Wv = W1[F:2 * F, :].astype(np.float32)    # neighbor part

    wv = np.zeros((F + 1, H), dtype=bf16)
    wv[0:F, :] = Wv.astype(bf16)
    wv[F, :] = b1.astype(bf16)

    wpp = Wp.astype(bf16)
    w2b = (W2.astype(np.float32) / np.float32(K)).astype(bf16)

    b2f = b2.astype(np.float32)
    b2hi = b2f.astype(bf16)
    b2lo = (b2f - b2hi.astype(np.float32)).astype(bf16)
    wxb = np.zeros((2, OUT), dtype=bf16)
    wxb[0, :] = b2hi
    wxb[1, :] = b2lo

    diag = np.zeros((128, 4, 128), dtype=bf16)
    for t in range(4):
        diag[np.arange(128), t, np.arange(128)] = bf16(-BIG)
    ident = np.eye(128, dtype=bf16)

    idrep = np.zeros((128, CH), dtype=bf16)
    for p in range(128):
        idrep[p, p * 16:(p + 1) * 16] = bf16(1.0)

    p128 = np.arange(128, dtype=np.float32)[:, None]
    jt4 = np.arange(4, dtype=np.float32)[None, :]
    iotn = (p128 + 128 * jt4).astype(np.float32)

    xt = np.ascontiguousarray(x.transpose(0, 2, 1).astype(np.float32))
    # host-side xtb (bf16 features + two ones rows)
    xtb = np.ones((B, F + 2, N), dtype=bf16)
    xtb[:, 0:F, :] = xt.astype(bf16)
    # host-side keys operands: hi/lo splits of coords and 0.5|c|^2
    c = xt[:, 0:2, :].astype(np.float32)                  # [B, 2, N]
    chi = c.astype(bf16)
    clo = (c - chi.astype(np.float32)).astype(bf16)
    sq = 0.5 * c * c
    sqhi = sq.astype(bf16)
    sqlo = (sq - sqhi.astype(np.float32)).astype(bf16)
    l10 = np.empty((B, 10, N), dtype=bf16)
    l10[:, 0:4, :] = bf16(-1.0)
    l10[:, 4, :] = chi[:, 0]
    l10[:, 5, :] = chi[:, 0]
    l10[:, 6, :] = clo[:, 0]
    l10[:, 7, :] = chi[:, 1]
    l10[:, 8, :] = chi[:, 1]
    l10[:, 9, :] = clo[:, 1]
    r10 = np.empty((B, 10, N), dtype=bf16)
    r10[:, 0, :] = sqhi[:, 0]
    r10[:, 1, :] = sqlo[:, 0]
    r10[:, 2, :] = sqhi[:, 1]
    r10[:, 3, :] = sqlo[:, 1]
    r10[:, 4, :] = chi[:, 0]
    r10[:, 5, :] = clo[:, 0]
    r10[:, 6, :] = chi[:, 0]
    r10[:, 7, :] = chi[:, 1]
    r10[:, 8, :] = clo[:, 1]
    r10[:, 9, :] = chi[:, 1]
    return xtb, l10, r10, wv, wpp, w2b, wxb, diag, ident, idrep, iotn


def _in_maps(xtb, l10, r10, wv, wpp, w2b, wxb, diag, ident, idrep, iotn):
    return [{
        "xtb": xtb[c * EV:(c + 1) * EV],
        "l10": l10[c * EV:(c + 1) * EV],
        "r10": r10[c * EV:(c + 1) * EV],
        "wv": wv, "wpp": wpp, "w2b": w2b, "wxb": wxb,
        "diag": diag, "ident": ident, "idrep": idrep, "iotn": iotn,
    } for c in range(NCORES)]


def kernel(x, W1, b1, W2, b2):
    from concourse.bass_utils import run_bass_kernel_spmd

    key = "nc"
    if key not in _cache:
        _cache[key] = _build_nc()
    nc = _cache[key]

    prepped = _prep_inputs(
        np.asarray(x), np.asarray(W1), np.asarray(b1),
        np.asarray(W2), np.asarray(b2))

    res = run_bass_kernel_spmd(nc, _in_maps(*prepped), list(range(NCORES)))
    outs = [res.results[c]["out"] for c in range(NCORES)]
    full = np.concatenate(outs, axis=0)
    return np.ascontiguousarray(full.transpose(0, 2, 1)).astype(np.float32)


# revision 23
# speedup vs baseline: 1.0628x; 1.0096x over previous
"""EdgeConv (ParticleNet-style) Trainium2 kernel, v5.

Per event (16/core), software-pipelined 3 deep:
  - keys[i, j] = c_i.c_j - 0.5|c_j|^2 via bf16 hi/lo-split matmul (10-row
    contraction, host-prepped l10/r10, ~fp32-exact) + 128-col -BIG diag MMs.
  - top-16 per row via DVE max8/match_replace/max_index (fp32).
  - edge order n = t*2048 + p'*16 + r  (center i = 128t + p', slot r).
  - idx u16 -> fp16 -> flat rep[0:1, 8192] (4 DMAs) -> doubling DMAs -> [128,
    8192]; one-hot OH[jlo, jt, n] = (rep == jlo + 128*jt) via DVE
    tensor_scalar is_equal (fp16 in, bf16 out, fast mode).
  - gather per 512-col PSUM segment: p-add (static identrep x pnT_t) then
    4 jt one-hot matmuls; scalar ACT Relu drains -> vgT bf16
    (= relu(p_i + v_j)).
  - layer 2: ops[o, i] = sum_r (W2/16)^T @ vgT_r + b2; cols already in
    natural i order.
"""

import numpy as np
import ml_dtypes

B, N, F = 128, 512, 32
K = 16
H, OUT = 128, 64
NCORES = 8
EV = B // NCORES
BIG = np.float32(1e30)
NE = K * N          # 8192 edges per event
CH = 2048           # chunk = one t-block of centers
SEG = 512           # PSUM-bank-sized matmul segment

_cache = {}


def _build_nc(n_ev=EV):
    import concourse.bass as bass
    import concourse.bacc as bacc
    import concourse.tile as tile
    import concourse.mybir as mybir
    from contextlib import ExitStack

    dt = mybir.dt
    AOT = mybir.AluOpType
    AF = mybir.ActivationFunctionType
    nc = bacc.Bacc("TRN2", target_bir_lowering=False, debug=False,
                   enable_asserts=False, num_devices=NCORES)

    xtb_d = nc.dram_tensor("xtb", [n_ev, F + 2, N], dt.bfloat16,
                           kind="ExternalInput")
    l10_d = nc.dram_tensor("l10", [n_ev, 10, N], dt.bfloat16,
                           kind="ExternalInput")
    r10_d = nc.dram_tensor("r10", [n_ev, 10, N], dt.bfloat16,
                           kind="ExternalInput")
    wv_d = nc.dram_tensor("wv", [F + 1, H], dt.bfloat16, kind="ExternalInput")
    wpp_d = nc.dram_tensor("wpp", [F, H], dt.bfloat16, kind="ExternalInput")
    w2b_d = nc.dram_tensor("w2b", [H, OUT], dt.bfloat16, kind="ExternalInput")
    wxb_d = nc.dram_tensor("wxb", [2, OUT], dt.bfloat16, kind="ExternalInput")
    diag_d = nc.dram_tensor("diag", [128, 4, 128], dt.bfloat16,
                            kind="ExternalInput")
    ident_d = nc.dram_tensor("ident", [128, 128], dt.bfloat16,
                             kind="ExternalInput")
    idrep_d = nc.dram_tensor("idrep", [128, CH], dt.bfloat16,
                             kind="ExternalInput")
    iotn_d = nc.dram_tensor("iotn", [128, 4], dt.float32, kind="ExternalInput")
    out_d = nc.dram_tensor("out", [n_ev, OUT, N], dt.float32,
                           kind="ExternalOutput")

    with tile.TileContext(nc) as tc, ExitStack() as ctx:
        cpool = ctx.enter_context(tc.tile_pool(name="consts", bufs=1))
        ident = cpool.tile([128, 128], dt.bfloat16)
        nc.sync.dma_start(ident[:], ident_d[:])
        diag = cpool.tile([128, 4, 128], dt.bfloat16)
        nc.sync.dma_start(diag[:], diag_d[:])
        wv = cpool.tile([F + 1, H], dt.bfloat16)
        nc.sync.dma_start(wv[:], wv_d[:])
        wpp = cpool.tile([F, H], dt.bfloat16)
        nc.sync.dma_start(wpp[:], wpp_d[:])
        w2b = cpool.tile([H, OUT], dt.bfloat16)
        nc.sync.dma_start(w2b[:], w2b_d[:])
        wxb = cpool.tile([2, OUT], dt.bfloat16)
        nc.sync.dma_start(wxb[:], wxb_d[:])
        idrep = cpool.tile([128, CH], dt.bfloat16)
        nc.sync.dma_start(idrep[:], idrep_d[:])
        iotn = cpool.tile([128, 4], dt.float32)
        nc.sync.dma_start(iotn[:], iotn_d[:])
        ones2 = cpool.tile([2, N], dt.bfloat16)
        nc.gpsimd.memset(ones2[:], 1.0)

        xtb_pool = ctx.enter_context(tc.tile_pool(name="xtb", bufs=3))
        l10_pool = ctx.enter_context(tc.tile_pool(name="l10", bufs=3))
        r10_pool = ctx.enter_context(tc.tile_pool(name="r10", bufs=3))
        kt_pool = ctx.enter_context(tc.tile_pool(name="kt", bufs=8))
        k2_pool = ctx.enter_context(tc.tile_pool(name="k2", bufs=2))
        vals_pool = ctx.enter_context(tc.tile_pool(name="vals", bufs=2))
        idxs_pool = ctx.enter_context(tc.tile_pool(name="idxs", bufs=2))
        idxf_pool = ctx.enter_context(tc.tile_pool(name="idxf", bufs=2))
        rep_pool = ctx.enter_context(tc.tile_pool(name="rep", bufs=3))
        vsb_pool = ctx.enter_context(tc.tile_pool(name="vsb", bufs=3))
        pn_pool = ctx.enter_context(tc.tile_pool(name="pn", bufs=3))
        oh_pool = ctx.enter_context(tc.tile_pool(name="oh", bufs=4))
        vgT_pool = ctx.enter_context(tc.tile_pool(name="vgT", bufs=2))
        osb_pool = ctx.enter_context(tc.tile_pool(name="osb", bufs=2))

        kps_pool = ctx.enter_context(
            tc.tile_pool(name="kps", bufs=2, space="PSUM"))
        vpn_pool = ctx.enter_context(
            tc.tile_pool(name="vpn", bufs=1, space="PSUM"))
        gps_pool = ctx.enter_context(
            tc.tile_pool(name="gps", bufs=3, space="PSUM"))
        ops_pool = ctx.enter_context(
            tc.tile_pool(name="ops", bufs=1, space="PSUM"))

        st = [None] * n_ev

        def stage_a(e):
            """Load host-prepped xtb / l10 / r10 for event e."""
            xtb = xtb_pool.tile([F + 2, N], dt.bfloat16)
            nc.sync.dma_start(xtb[:], xtb_d[e])
            l10 = l10_pool.tile([10, N], dt.bfloat16)
            nc.sync.dma_start(l10[:], l10_d[e])
            r10 = r10_pool.tile([10, N], dt.bfloat16)
            nc.sync.dma_start(r10[:], r10_d[e])
            st[e] = {"xtb": xtb, "l10": l10, "r10": r10}

        def stage_b(e):
            """keys/v/pn matmuls + PSUM drains for event e."""
            s = st[e]
            xtb, l10, r10 = s["xtb"], s["l10"], s["r10"]
            kts = []
            for t in range(4):
                kps = kps_pool.tile([128, N], dt.float32)
                nc.tensor.matmul(kps[:], l10[:, 128 * t:128 * (t + 1)],
                                 r10[:], start=True, stop=False)
                nc.tensor.matmul(kps[:, 128 * t:128 * (t + 1)], ident[:],
                                 diag[:, t, :], start=False, stop=True,
                                 skip_group_check=True)
                kt = kt_pool.tile([128, N], dt.float32)
                nc.scalar.activation(kt[:], kps[:], AF.Copy)
                kts.append(kt)
            vps = vpn_pool.tile([128, N], dt.float32, name="vps")
            for c in range(4):
                nc.tensor.matmul(vps[:, H * c:H * (c + 1)],
                                 xtb[0:F + 1, 128 * c:128 * (c + 1)],
                                 wv[:], start=True, stop=True)
            v_sb = vsb_pool.tile([128, 4, H], dt.bfloat16)
            nc.scalar.activation(v_sb[:].opt(), vps[:], AF.Copy)
            pnp = vpn_pool.tile([128, N], dt.float32, name="pnp")
            for c in range(4):
                nc.tensor.matmul(pnp[:, H * c:H * (c + 1)],
                                 xtb[0:F, 128 * c:128 * (c + 1)],
                                 wpp[:], start=True, stop=True)
            pn_sb = pn_pool.tile([128, 4, H], dt.bfloat16)
            nc.scalar.activation(pn_sb[:].opt(), pnp[:], AF.Copy)
            s.update(kts=kts, v_sb=v_sb, pn_sb=pn_sb)

        def stage_d(e):
            """Selection + idx assembly + rep replication for event e."""
            s = st[e]
            vals = vals_pool.tile([128, 16], dt.float32)
            idxs = idxs_pool.tile([128, 64], dt.uint16)
            for t in range(4):
                kt = s["kts"][t][:]
                k2 = k2_pool.tile([128, N], dt.float32)
                v0 = vals[:, 0:8]
                v1 = vals[:, 8:16]
                nc.vector.max(v0, kt)
                nc.vector.match_replace(k2[:], v0, kt, -float(BIG))
                nc.vector.max(v1, k2[:])
                nc.vector.max_index(idxs[:, 16 * t:16 * t + 8], v0, kt)
                nc.vector.max_index(idxs[:, 16 * t + 8:16 * t + 16],
                                    v1, k2[:])
            idxf = idxf_pool.tile([128, 64], dt.float16)
            nc.vector.tensor_copy(idxf[:], idxs[:])
            rep = rep_pool.tile([128, NE], dt.float16)
            # dst col n = t*2048 + p*16 + r  <-  idxf[p, 16t + r]
            for t in range(4):
                nc.sync.dma_start(
                    rep[0:1, CH * t:CH * (t + 1)].rearrange(
                        "q (p r) -> q p r", p=128, r=16),
                    idxf[:, 16 * t:16 * (t + 1)])
            w = 1
            while w < 128:
                nc.sync.dma_start(rep[w:2 * w, :], rep[0:w, :])
                w *= 2
            s.update(rep=rep)

        def stage_oh(e, trange):
            """Build one-hot tiles for chunks `trange` of event e."""
            s = st[e]
            rep = s["rep"]
            ohs = s.setdefault("ohs", {})
            for t in trange:
                csl = slice(CH * t, CH * (t + 1))
                oh = oh_pool.tile([128, 4, CH], dt.bfloat16)
                for jt in range(4):
                    nc.vector.tensor_scalar(
                        oh[:, jt, :].opt(), rep[:, csl],
                        iotn[:, jt:jt + 1], None, op0=AOT.is_equal)
                ohs[t] = oh

        def stage_g(e, trange):
            """Gather + relu drain (+ layer 2 + out) for event e."""
            s = st[e]
            v_sb, pn_sb = s["v_sb"], s["pn_sb"]
            if "vgT" not in s:
                s["vgT"] = vgT_pool.tile([128, NE], dt.bfloat16, name="vgT")
            vgT = s["vgT"]
            for t in trange:
                oh = s["ohs"].pop(t)
                for sg in range(CH // SEG):
                    gps = gps_pool.tile([128, SEG], dt.float32, name="gps")
                    nc.tensor.matmul(gps[:], pn_sb[:, t, :],
                                     idrep[:, SEG * sg:SEG * (sg + 1)],
                                     start=True, stop=False)
                    for jt in range(4):
                        nc.tensor.matmul(
                            gps[:], v_sb[:, jt, :],
                            oh[:, jt, SEG * sg:SEG * (sg + 1)].opt(),
                            start=False, stop=(jt == 3))
                    nc.scalar.activation(
                        vgT[:, CH * t + SEG * sg:CH * t + SEG * (sg + 1)],
                        gps[:], AF.Relu)
            if trange[-1] != 3:
                return
            ops = ops_pool.tile([OUT, N], dt.float32)
            vg4 = vgT[:].rearrange("h (t p r) -> h r t p", t=4, p=128, r=16)
            for r in range(K):
                nc.tensor.matmul(ops[:], w2b[:], vg4[:, r].opt(),
                                 start=(r == 0), stop=False)
            nc.tensor.matmul(ops[:], wxb[:], ones2[:],
                             start=False, stop=True)
            osb = osb_pool.tile([OUT, N], dt.float32)
            nc.scalar.activation(osb[:], ops[:], AF.Copy)
            nc.sync.dma_start(out_d[e], osb[:])
            st[e] = None

        # 3-deep software pipeline.  OH for chunks 0-1 of an event is built
        # one iteration ahead of its gather, so the PE enters the gather
        # block with its feed already materialized; OH 2-3 is built right
        # after selection, just in time for the second gather half.
        stage_a(0)
        if n_ev > 1:
            stage_a(1)
        for e in range(n_ev):
            stage_b(e)
            if e + 2 < n_ev:
                stage_a(e + 2)
            if e >= 2:
                stage_g(e - 2, [0, 1])
            stage_d(e)
            if e >= 2:
                stage_oh(e - 2, [2, 3])
                stage_g(e - 2, [2, 3])
            if e >= 1:
                stage_oh(e - 1, [0, 1])
        e = n_ev - 2
        stage_g(e, [0, 1])
        stage_oh(e, [2, 3])
        stage_g(e, [2, 3])
        e = n_ev - 1
        stage_oh(e, [0, 1])
        stage_g(e, [0, 1])
        stage_oh(e, [2, 3])
        stage_g(e, [2, 3])

    nc.compile()
    return nc


def _prep_inputs(x, W1, b1, W2, b2):
    bf16 = ml_dtypes.bfloat16
    W1a = W1[0:F, :].astype(np.float64)
    W1b = W1[F:2 * F, :].astype(np.float64)
    Wp = W1a - W1b                            # center part
    Wv = W1[F:2 * F, :].astype(np.float32)    # neighbor part

    wv = np.zeros((F + 1, H), dtype=bf16)
    wv[0:F, :] = Wv.astype(bf16)
    wv[F, :] = b1.astype(bf16)

    wpp = Wp.astype(bf16)
    w2b = (W2.astype(np.float32) / np.float32(K)).astype(bf16)

    b2f = b2.astype(np.float32)
    b2hi = b2f.astype(bf16)
    b2lo = (b2f - b2hi.astype(np.float32)).astype(bf16)
    wxb = np.zeros((2, OUT), dtype=bf16)
    wxb[0, :] = b2hi
    wxb[1, :] = b2lo

    diag = np.zeros((128, 4, 128), dtype=bf16)
    for t in range(4):
        diag[np.arange(128), t, np.arange(128)] = bf16(-BIG)
    ident = np.eye(128, dtype=bf16)

    idrep = np.zeros((128, CH), dtype=bf16)
    for p in range(128):
        idrep[p, p * 16:(p + 1) * 16] = bf16(1.0)

    p128 = np.arange(128, dtype=np.float32)[:, None]
    jt4 = np.arange(4, dtype=np.float32)[None, :]
    iotn = (p128 + 128 * jt4).astype(np.float32)

    xt = np.ascontiguousarray(x.transpose(0, 2, 1).astype(np.float32))
    # host-side xtb (bf16 features + two ones rows)
    xtb = np.ones((B, F + 2, N), dtype=bf16)
    xtb[:, 0:F, :] = xt.astype(bf16)
    # host-side keys operands: hi/lo splits of coords and 0.5|c|^2
    c = xt[:, 0:2, :].astype(np.float32)                  # [B, 2, N]
    chi = c.astype(bf16)
    clo = (c - chi.astype(np.float32)).astype(bf16)
    sq = 0.5 * c * c
    sqhi = sq.astype(bf16)
    sqlo = (sq - sqhi.astype(np.float32)).astype(bf16)
    l10 = np.empty((B, 10, N), dtype=bf16)
    l10[:, 0:4, :] = bf16(-1.0)
    l10[:, 4, :] = chi[:, 0]
    l10[:, 5, :] = chi[:, 0]
    l10[:, 6, :] = clo[:, 0]
    l10[:, 7, :] = chi[:, 1]
    l10[:, 8, :] = chi[:, 1]
    l10[:, 9, :] = clo[:, 1]
    r10 = np.empty((B, 10, N), dtype=bf16)
    r10[:, 0, :] = sqhi[:, 0]
    r10[:, 1, :] = sqlo[:, 0]
    r10[:, 2, :] = sqhi[:, 1]
    r10[:, 3, :] = sqlo[:, 1]
    r10[:, 4, :] = chi[:, 0]
    r10[:, 5, :] = clo[:, 0]
    r10[:, 6, :] = chi[:, 0]
    r10[:, 7, :] = chi[:, 1]
    r10[:, 8, :] = clo[:, 1]
    r10[:, 9, :] = chi[:, 1]
    return xtb, l10, r10, wv, wpp, w2b, wxb, diag, ident, idrep, iotn


def _in_maps(xtb, l10, r10, wv, wpp, w2b, wxb, diag, ident, idrep, iotn):
    return [{
        "xtb": xtb[c * EV:(c + 1) * EV],
        "l10": l10[c * EV:(c + 1) * EV],
        "r10": r10[c * EV:(c + 1) * EV],
        "wv": wv, "wpp": wpp, "w2b": w2b, "wxb": wxb,
        "diag": diag, "ident": ident, "idrep": idrep, "iotn": iotn,
    } for c in range(NCORES)]


def kernel(x, W1, b1, W2, b2):
    from concourse.bass_utils import run_bass_kernel_spmd

    key = "nc"
    if key not in _cache:
        _cache[key] = _build_nc()
    nc = _cache[key]

    prepped = _prep_inputs(
        np.asarray(x), np.asarray(W1), np.asarray(b1),
        np.asarray(W2), np.asarray(b2))

    res = run_bass_kernel_spmd(nc, _in_maps(*prepped), list(range(NCORES)))
    outs = [res.results[c]["out"] for c in range(NCORES)]
    full = np.concatenate(outs, axis=0)
    return np.ascontiguousarray(full.transpose(0, 2, 1)).astype(np.float32)
